# revision 2
# baseline (speedup 1.0000x reference)
"""Chamfer loss (sqrt variant) Trainium2 Bass kernel.

Reference computation (per batch b of 32):
    d[n, m]   = ||points[b, n] - gts[b, m]||^2          (4096 x 4096)
    p2g[b]    = mean_n sqrt(relu(min_m d[n, m]))
    g2p[b]    = mean_m sqrt(relu(min_n d[n, m]))
    returns (mean_b(p2g + g2p), mean_b p2g, mean_b g2p)

Strategy (8 NeuronCores, data-parallel over batch, 4 batches/core):
  * One K=24 bf16 triple-split augmented matmul per batch emits
    e = -d = -|p|^2 - |g|^2 + 2 p.g  directly into PSUM, tiled
    [128 points x 2048 gts].  The 3-term hi/mid/lo split keeps the
    (catastrophic cancellation sensitive) distance accurate to ~2^-24
    of the point norms while running the PE at full bf16 rate.
  * ScalarE exits each PSUM tile once (activation Copy, cast to bf16
    SBUF).  DVE is the bottleneck engine and is kept to the minimum op
    set, all at the bf16 2x_1p rate: one running column max per chunk
    (g2p direction), a 4-level pairwise fold tree per chunk down to 256
    (p2g direction), and one batched 1x tensor_reduce per 8 chunks.
    Chunk 0 seeds the running max via tensor_scalar_max (4x_2p rate),
    which also removes the per-batch memset.
  * GPSIMD partition_all_reduce folds the [128, 4096] running tile
    across partitions (g2p) and the final row sums (p2g).
  * Per-batch tail ops (clamp/sqrt/sum chains on [1, 4096]) are
    emitted interleaved into the NEXT batch's chunk loop (after chunks
    2 and 5) so the in-order DVE/Act queues never stall on the Pool
    all-reduce latency.
Host side only reshapes/shards inputs and averages the 8x4x2 per-batch
means into the 3 output scalars.
"""

import os
import sys

import numpy as np

for _p in ("/opt/trn_rl_repo",):
    if _p not in sys.path and os.path.isdir(_p):
        sys.path.insert(0, _p)

import ml_dtypes

import concourse.bacc as bacc
import concourse.tile as tile
from concourse import bass_isa, mybir

BF16 = ml_dtypes.bfloat16

N_CORES = 8
NPTS = 4096
NBATCH_TOTAL = 32


def build_nc(nbatch: int, npts: int, reps: int = 1):
    """Build (and bacc-compile) the per-core Bass program.

    Inputs (per core):
      sform [nbatch, 24, npts] bf16 -- stationary forms (points side)
      vform [nbatch, 24, npts] bf16 -- moving forms (gts side)
    Output:
      res [1, 2*nbatch] f32 -- per batch: (mean sqrt min_m d, mean sqrt min_n d)
    """
    f32 = mybir.dt.float32
    bf16 = mybir.dt.bfloat16
    K = 24
    P = 128
    nchunks = npts // P
    F = min(2048, npts)  # PSUM tile free size (4 banks; filled by N=512 matmuls)
    nq = npts // F       # halves of the moving axis
    RED = min(8, nchunks)  # chunks per batched row-reduce
    assert nchunks % RED == 0
    NEG = -3.0e38

    nc = bacc.Bacc("TRN2", target_bir_lowering=False, debug=False)

    sform = nc.dram_tensor("sform", [nbatch, K, npts], bf16, kind="ExternalInput").ap()
    vform = nc.dram_tensor("vform", [nbatch, K, npts], bf16, kind="ExternalInput").ap()
    res_d = nc.dram_tensor("res", [1, 2 * nbatch], f32, kind="ExternalOutput").ap()

    with tile.TileContext(nc) as tc:
        with (
            tc.tile_pool(name="io", bufs=2) as io_pool,
            tc.tile_pool(name="run", bufs=2) as run_pool,
            tc.tile_pool(name="work", bufs=2) as work_pool,
            tc.tile_pool(name="tail", bufs=2) as tail_pool,
            tc.tile_pool(name="small", bufs=1) as small_pool,
            tc.tile_pool(name="psum", bufs=2, space="PSUM") as psum_pool,
        ):
            res_sb = small_pool.tile([1, 2 * nbatch], f32)

            def emit_tail_a(ctx):
                b, rg, cmax = ctx
                # g2p: partition max of the running column max, then clamp+sqrt
                ar = tail_pool.tile([P, npts], bf16, tag="ar")
                nc.gpsimd.partition_all_reduce(ar, rg, P, bass_isa.ReduceOp.max)
                g2 = tail_pool.tile([1, npts], bf16, tag="g2")
                g1 = tail_pool.tile([1, npts], bf16, tag="g1")
                nc.vector.tensor_scalar_min(g1, ar[0:1, :], 0.0)
                nc.scalar.activation(
                    out=g2, in_=g1,
                    func=mybir.ActivationFunctionType.Sqrt,
                    scale=-1.0,
                )
                # p2g: clamp + sqrt of per-chunk row maxes
                sq = tail_pool.tile([P, nchunks], f32, tag="sq")
                nc.vector.tensor_scalar_min(cmax, cmax, 0.0)
                nc.scalar.activation(
                    out=sq, in_=cmax,
                    func=mybir.ActivationFunctionType.Sqrt,
                    scale=-1.0,
                )
                ctx.append((ar, g1, g2, sq))

            def emit_tail_b(ctx):
                b, rg, cmax, (ar, g1, g2, sq) = ctx
                # g2p: fold-add the sqrt'd column mins, reduce, scale
                w = npts
                while w > 512:
                    w //= 2
                    nc.vector.tensor_add(
                        g2[0:1, 0:w], g2[0:1, 0:w], g2[0:1, w:2 * w]
                    )
                gacc = tail_pool.tile([1, 1], f32, tag="gacc")
                nc.vector.tensor_reduce(
                    out=gacc, in_=g2[0:1, 0:w],
                    axis=mybir.AxisListType.X, op=mybir.AluOpType.add,
                )
                nc.scalar.mul(
                    res_sb[0:1, 2 * b + 1:2 * b + 2], gacc[0:1, 0:1], 1.0 / npts
                )
                # p2g: row-sum, partition sum, scale
                rowsum = tail_pool.tile([P, 1], f32, tag="rowsum")
                nc.vector.tensor_reduce(
                    out=rowsum, in_=sq,
                    axis=mybir.AxisListType.X, op=mybir.AluOpType.add,
                )
                psum_all = tail_pool.tile([P, 1], f32, tag="psum_all")
                nc.gpsimd.partition_all_reduce(
                    psum_all, rowsum, P, bass_isa.ReduceOp.add
                )
                nc.scalar.mul(
                    res_sb[0:1, 2 * b:2 * b + 1], psum_all[0:1, 0:1], 1.0 / npts
                )

            pend = None
            for b in [b for _ in range(reps) for b in range(nbatch)]:
                s_t = io_pool.tile([K, npts], bf16, tag="s")
                v_t = io_pool.tile([K, npts], bf16, tag="v")
                nc.sync.dma_start(out=s_t, in_=sform[b])
                nc.sync.dma_start(out=v_t, in_=vform[b])

                # running column-max of e = -d (per g index), over all chunks
                rg = run_pool.tile([P, npts], bf16, tag="rg")
                # per-chunk row-max accumulators
                cmax = work_pool.tile([P, nchunks], f32, tag="cmax")
                # fold-to-256 staging for RED chunks at a time
                cstage = work_pool.tile([P, RED, 256], bf16, tag="cstage")

                for c in range(nchunks):
                    if c == 2 and pend is not None:
                        emit_tail_a(pend)
                    if c == 5 and pend is not None:
                        emit_tail_b(pend)
                        pend = None
                    lhs = s_t[0:K, c * P:(c + 1) * P]
                    # one bf16 [128, npts] staging tile per chunk
                    ebf = work_pool.tile([P, npts], bf16, tag="ebf", bufs=3)
                    for q in range(nq):
                        ps = psum_pool.tile([P, F], f32, tag="ps")
                        for j in range(0, F, 512):
                            w = min(512, F - j)
                            nc.tensor.matmul(
                                ps[:, j:j + w],
                                lhs,
                                v_t[0:K, q * F + j:q * F + j + w],
                                start=True, stop=True,
                            )
                        # single PSUM exit: ScalarE cast-copy to bf16 SBUF
                        nc.scalar.copy(ebf[:, q * F:(q + 1) * F], ps)
                    # running column max, one full-width op per chunk
                    if c == 0:
                        nc.vector.tensor_scalar_max(rg, ebf, NEG)
                    else:
                        nc.vector.tensor_max(rg, rg, ebf)
                    # direction 1: in-place pairwise folds down to 512 ...
                    w = npts
                    while w > 512:
                        w //= 2
                        nc.vector.tensor_max(
                            ebf[:, 0:w], ebf[:, 0:w], ebf[:, w:2 * w]
                        )
                    # ... then 512->256 into the staging slot ...
                    nc.vector.tensor_max(
                        cstage[:, c % RED], ebf[:, 0:256], ebf[:, 256:512]
                    )
                    # ... and one batched reduce per RED chunks
                    if c % RED == RED - 1:
                        nc.vector.tensor_reduce(
                            out=cmax[:, c - (RED - 1):c + 1],
                            in_=cstage,
                            axis=mybir.AxisListType.X,
                            op=mybir.AluOpType.max,
                        )

                pend = [b, rg, cmax]

            emit_tail_a(pend)
            emit_tail_b(pend)

            nc.sync.dma_start(out=res_d, in_=res_sb)

    nc.compile()
    return nc


def _split3(x64):
    h1 = x64.astype(np.float32).astype(BF16)
    r1 = x64 - h1.astype(np.float64)
    h2 = r1.astype(np.float32).astype(BF16)
    r2 = r1 - h2.astype(np.float64)
    h3 = r2.astype(np.float32).astype(BF16)
    return h1, h2, h3


def make_forms(pts: np.ndarray, gts: np.ndarray):
    """Host-side sharding-layout prep for ONE batch.

    pts, gts: [npts, 3] float32.
    Returns (S, V) each [24, npts] bf16 such that for the augmented matmul
    S(p_col n) . V(g_col m) = -d[n, m] up to ~2^-24 of the point norms.
    Covers hi/mid/lo product pairs (1,1),(2,1),(3,1),(1,2),(2,2),(1,3).
    """
    out = []
    for x in (pts, gts):
        x64 = np.asarray(x, dtype=np.float64)
        X1, X2, X3 = _split3(x64)
        n2 = (x64 ** 2).sum(-1)
        N1, N2, N3 = _split3(n2)
        out.append((X1, X2, X3, N1, N2, N3))
    (P1, P2, P3, A1, A2, A3), (G1, G2, G3, B1, B2, B3) = out

    npts = pts.shape[0]
    one = np.ones(npts, BF16)
    two = BF16(2.0)

    def rows3(M):
        return [M[:, 0], M[:, 1], M[:, 2]]

    S = np.stack(
        [-one, -one, -one]
        + rows3(two * P1) + rows3(two * P2) + rows3(two * P3)
        + rows3(two * P1) + rows3(two * P2)
        + rows3(two * P1)
        + [-A1, -A2, -A3])
    V = np.stack(
        [B1, B2, B3]
        + rows3(G1) + rows3(G1) + rows3(G1)
        + rows3(G2) + rows3(G2)
        + rows3(G3)
        + [one, one, one])
    return np.ascontiguousarray(S), np.ascontiguousarray(V)


_NC_CACHE = {}


def _get_nc(nbatch, npts):
    key = (nbatch, npts)
    if key not in _NC_CACHE:
        _NC_CACHE[key] = build_nc(nbatch, npts)
    return _NC_CACHE[key]


def kernel(points: np.ndarray, gts: np.ndarray):
    points = np.asarray(points, dtype=np.float32)
    gts = np.asarray(gts, dtype=np.float32)
    nb, npts, _ = points.shape
    assert nb % N_CORES == 0
    bpc = nb // N_CORES

    nc = _get_nc(bpc, npts)

    in_maps = []
    for r in range(N_CORES):
        S = np.empty((bpc, 24, npts), BF16)
        V = np.empty((bpc, 24, npts), BF16)
        for i in range(bpc):
            b = r * bpc + i
            S[i], V[i] = make_forms(points[b], gts[b])
        in_maps.append({"sform": S, "vform": V})

    from concourse.bass_utils import run_bass_kernel_spmd

    br = run_bass_kernel_spmd(nc, in_maps, core_ids=list(range(N_CORES)))
    results = br.results

    vals = np.stack(
        [np.asarray(results[r]["res"]).reshape(bpc, 2) for r in range(N_CORES)]
    )  # [cores, bpc, 2]
    p2g = vals[..., 0].astype(np.float64).mean()
    g2p = vals[..., 1].astype(np.float64).mean()
    loss = p2g + g2p
    return (np.float32(loss), np.float32(p2g), np.float32(g2p))


if __name__ == "__main__":
    rng = np.random.default_rng(0)
    pts = rng.standard_normal((NBATCH_TOTAL, NPTS, 3), dtype=np.float32)
    gt = rng.standard_normal((NBATCH_TOTAL, NPTS, 3), dtype=np.float32)
    print(kernel(pts, gt))
